# revision 27
# baseline (speedup 1.0000x reference)
"""Causal multi-head self-attention on 8 Trainium2 NeuronCores.

Sharding: tensor-parallel over heads. Each of the 8 cores owns 2 heads
(128 of the 1024 qkv dims). Per core:
  - QT/KT = (x @ Wq_c^T)^T etc. computed in transposed-activation layout
    [128 dims, 8192 tokens] (Wq pre-scaled by 1/sqrt(hd) on host).
  - V transposed back to natural [tokens, dims] via PE transpose, with a
    ones-column appended so the attn@V matmul also produces the softmax
    denominator (softmax computed without max-subtraction: scores are
    O(+-10) so exp() is safe in fp32).
  - scores^T = K Q^T per (batch, head), causal via per-tile widths + one
    128x128 staircase mask on the diagonal tiles. The two heads' score
    matmuls run concurrently as row-tiled PE pairs (contraction 64 at
    base partitions 0/64 -> tile_position auto-derives).
  - out_partial = A_c^T @ Wo_c^T written per core; host sums the 8
    partials (the row-parallel all-reduce done on host).

All matmul operands are bf16; accumulation and softmax denominators
stay fp32 in PSUM. Output partials are written bf16, summed fp32 on
host.

Scheduling: the attention inner loop is ScalarE-bound (one exp per
128-key tile, ~1.15us each), so the Q/K/V projections of batch b+1 and
the output projections of the previous q-chunk are sliced into small
"filler" units and interleaved into batch b's attention stream at
k-tile granularity. Scores are software-pipelined one k-tile ahead of
attn@V so a pending exp never head-of-line-blocks the next score
matmul in the PE queue. This keeps the PE dense (HAM stays at full
clock) and hides the projection phases under the exp stream.
"""

import numpy as np
from collections import deque
from contextlib import ExitStack

import concourse.bass as bass
import concourse.mybir as mybir
import concourse.tile as tile
from concourse import bacc

F32 = mybir.dt.float32
F32R = mybir.dt.float32r
BF16 = mybir.dt.bfloat16
EXP = mybir.ActivationFunctionType.Exp
MULT = mybir.AluOpType.mult


class Cfg:
    def __init__(self, B=4, S=2048, D=1024, TCH=512, QCH=512, mm_dt="bf16"):
        self.B, self.S, self.D = B, S, D
        self.T = B * S
        self.KT = D // 128          # contraction tiles for projections
        self.TCH = TCH              # token chunk for projections
        self.QCH = QCH              # query chunk for attention
        self.NQC = S // QCH         # q chunks per batch
        self.HD = 64
        self.mm_dt = mm_dt
        assert S % QCH == 0 and QCH % 128 == 0 and self.T % TCH == 0


def _mmdt(cfg):
    return {"f32r": F32R, "bf16": BF16, "f32": F32}[cfg.mm_dt]


def build_program(cfg: Cfg):
    """Build the SPMD single-core Bass program (same program all cores)."""
    nc = bacc.Bacc("TRN2", target_bir_lowering=False, debug=False)
    B, S, D, T, KT = cfg.B, cfg.S, cfg.D, cfg.T, cfg.KT
    TCH, QCH, NQC = cfg.TCH, cfg.QCH, cfg.NQC
    NVT = T // 128                 # number of 128-token V tiles
    MMDT = _mmdt(cfg)
    OC = min(512, D)

    xT_d = nc.dram_tensor("xT", [128, KT, T], MMDT, kind="ExternalInput")
    wq_d = nc.dram_tensor("wq", [128, KT, 128], MMDT, kind="ExternalInput")
    wk_d = nc.dram_tensor("wk", [128, KT, 128], MMDT, kind="ExternalInput")
    wv_d = nc.dram_tensor("wv", [128, KT, 128], MMDT, kind="ExternalInput")
    wo_d = nc.dram_tensor("wo", [128, D], MMDT, kind="ExternalInput")
    mask_d = nc.dram_tensor("mask", [128, 128], MMDT, kind="ExternalInput")
    ident_d = nc.dram_tensor("ident", [128, 128], MMDT, kind="ExternalInput")
    out_d = nc.dram_tensor("out_p", [T, D], MMDT, kind="ExternalOutput")
    out_r = out_d.rearrange("(n p) o -> p n o", p=128)   # [128, NVT, D]

    with tile.TileContext(nc) as tc, ExitStack() as ctx:
        persist = ctx.enter_context(tc.tile_pool(name="persist", bufs=1))
        xp = ctx.enter_context(tc.tile_pool(name="xp", bufs=4))
        vtp = ctx.enter_context(tc.tile_pool(name="vtp", bufs=2))
        ptp = ctx.enter_context(tc.tile_pool(name="ptp", bufs=3))
        rcp = ctx.enter_context(tc.tile_pool(name="rcp", bufs=6))
        op = ctx.enter_context(tc.tile_pool(name="op", bufs=3))
        drp = ctx.enter_context(tc.tile_pool(name="drp", bufs=6, space="DRAM"))
        # PSUM budget (8 banks): scores 2x[128,1024]f32 = 4, att 2x1 = 2,
        # shared proj/transpose/outproj pool 2x1 = 2.
        scp = ctx.enter_context(tc.tile_pool(name="scp", bufs=2, space="PSUM"))
        attp = ctx.enter_context(
            tc.tile_pool(name="attp", bufs=2, space="PSUM"))
        mp = ctx.enter_context(tc.tile_pool(name="mp", bufs=2, space="PSUM"))

        qt_sb = persist.tile([128, T], MMDT, tag="qt")
        kt_sb = persist.tile([128, T], MMDT, tag="kt")
        a_sb = persist.tile([128, T], MMDT, tag="a")
        # V natural layout, one ones-column per head so each attn@V matmul
        # also emits the softmax denominator in its last output row:
        #   cols 0:64 = head0 dims, col 64 = 1.0,
        #   cols 65:129 = head1 dims, col 129 = 1.0
        v_sb = persist.tile([128, NVT, 130], MMDT, tag="v")
        wq_sb = persist.tile([128, KT, 128], MMDT, tag="wq")
        wk_sb = persist.tile([128, KT, 128], MMDT, tag="wk")
        wv_sb = persist.tile([128, KT, 128], MMDT, tag="wv")
        wo_sb = persist.tile([128, D], MMDT, tag="wo")
        mask_sb = persist.tile([128, 128], MMDT, tag="mask")
        ident = persist.tile([128, 128], MMDT, tag="ident")
        ones128 = persist.tile([128, 1], MMDT, tag="ones128")
        nc.vector.memset(ones128[:], 1.0)

        nc.sync.dma_start(wq_sb[:], wq_d[:])
        nc.sync.dma_start(wk_sb[:], wk_d[:])
        nc.sync.dma_start(wv_sb[:], wv_d[:])
        nc.sync.dma_start(wo_sb[:], wo_d[:])
        nc.sync.dma_start(mask_sb[:], mask_d[:])
        nc.sync.dma_start(ident[:], ident_d[:])
        nc.vector.tensor_copy(
            v_sb[:, :, 64:65],
            ones128[:, None, :].to_broadcast((128, NVT, 1)))
        nc.vector.tensor_copy(
            v_sb[:, :, 129:130],
            ones128[:, None, :].to_broadcast((128, NVT, 1)))

        # ---------------- projection filler units ----------------------
        def proj_chunk_units(tci):
            """Yield closures; each emits ~0.2-0.5us of PE work projecting
            token chunk tci into qt/kt/v."""
            t0 = tci * TCH
            x_t = xp.tile([128, KT, TCH], MMDT, tag="x")
            nsplit = 4 if tci == 0 else 2
            step = KT // nsplit

            def load(si):
                nc.sync.dma_start(
                    x_t[:, si * step:(si + 1) * step, :],
                    xT_d[:, si * step:(si + 1) * step, t0:t0 + TCH])
            for si in range(nsplit):
                load(si)

            for w_sb, kind in ((wq_sb, "q"), (wk_sb, "k"), (wv_sb, "v")):
                ps_box = []

                def mk_mm(kt0, w_sb=w_sb, ps_box=ps_box):
                    def unit():
                        if not ps_box:
                            ps_box.append(mp.tile([128, TCH], F32, tag="mp", name="ps"))
                        ps = ps_box[0]
                        for kt in (kt0, kt0 + 1):
                            nc.tensor.matmul(
                                ps[:], w_sb[:, kt, :], x_t[:, kt, :],
                                start=(kt == 0), stop=(kt == KT - 1))
                    return unit
                for kt0 in range(0, KT, 2):
                    yield mk_mm(kt0)

                def mk_evac(kind=kind, ps_box=ps_box):
                    def unit():
                        ps = ps_box[0]
                        if kind == "q":
                            nc.scalar.copy(qt_sb[:, t0:t0 + TCH], ps[:])
                        elif kind == "k":
                            nc.scalar.copy(kt_sb[:, t0:t0 + TCH], ps[:])
                        else:
                            vt_t = vtp.tile([128, TCH], MMDT, tag="vt", name="vt")
                            ps_box.append(vt_t)
                            nc.vector.tensor_copy(vt_t[:], ps[:])
                    return unit
                yield mk_evac()

                if kind == "v":
                    def mk_tr(j, ps_box=ps_box):
                        def unit():
                            vt_t = ps_box[1]
                            tr = mp.tile([128, 128], MMDT, tag="mp", name="tr")
                            nc.tensor.transpose(
                                tr[:], vt_t[:, j * 128:(j + 1) * 128],
                                ident[:])
                            ktg = (t0 + j * 128) // 128
                            nc.vector.tensor_copy(
                                v_sb[:, ktg, 0:64], tr[:, 0:64])
                            nc.vector.tensor_copy(
                                v_sb[:, ktg, 65:129], tr[:, 64:128])
                        return unit
                    for j in range(TCH // 128):
                        yield mk_tr(j)

        # ---------------- output-projection filler units ----------------
        def outproj_units(t0, tail=False):
            """Yield closures for the output projection of QCH tokens
            starting at t0 (one 128-token tile per unit pair). In the
            kernel tail the PSUM evacuations alternate DVE/ACT (ACT is
            idle there) to double the drain rate."""
            for ti in range(QCH // 128):
                tt = t0 // 128 + ti
                o_box = []

                def mk_oc(oc, ti=ti, tt=tt, o_box=o_box):
                    def unit():
                        if not o_box:
                            o_box.append(op.tile([128, D], MMDT, tag="osb", name="osb"))
                        o_sb = o_box[0]
                        ps = mp.tile([128, OC], F32, tag="mp", name="wops")
                        nc.tensor.matmul(
                            ps[:],
                            a_sb[:, tt * 128:(tt + 1) * 128],
                            wo_sb[:, oc * OC:(oc + 1) * OC],
                            start=True, stop=True)
                        if tail and (ti + oc) % 2:
                            nc.scalar.copy(
                                o_sb[:, oc * OC:(oc + 1) * OC], ps[:])
                        else:
                            nc.vector.tensor_copy(
                                o_sb[:, oc * OC:(oc + 1) * OC], ps[:])
                        if oc == D // OC - 1:
                            nc.sync.dma_start(out_r[:, tt, :], o_sb[:])
                    return unit
                for oc in range(D // OC):
                    yield mk_oc(oc)

        urgq = deque()          # own-batch proj: deadline = end of chunk
        nxtq = deque()          # next-batch proj: deadline = end of batch
        normq = deque()         # deferred softmax-normalize (ua/ub units)
        outq = deque()          # outproj only; fed by drained ub units
        rem_batch = [0]         # usable k-tiles left in current batch
        rem_total = [sum(max((i * QCH + QCH) // 128 - 2, 1)
                         for i in range(NQC)) * B]
        reserve = [0]           # outq units held back for late batches
        cur_b = [0]

        def drain(q, n):
            for _ in range(n):
                if not q:
                    return
                q.popleft()()

        # ---------------- attention chunk -------------------------------
        def attention_chunk(b, qc, prev_finish=None):
            """Scores + softmax + attn@V + normalization for one
            (batch, q-chunk), with scores pipelined one k-tile ahead and
            filler drained between k-tiles."""
            base = b * S
            vbase = base // 128
            q0 = qc * QCH
            n_kt = (q0 + QCH) // 128
            att0 = attp.tile([65, QCH], F32, tag="att")
            att1 = attp.tile([65, QCH], F32, tag="att")

            # pacing: urgent proj finishes within this chunk, next-batch
            # proj within the current batch, outproj/normalize within a
            # ~24-k-tile horizon (bounds the end-of-kernel backlog). The
            # last 2 k-tiles of a chunk drain nothing, so the att-PSUM
            # evacuations land at the head of the DVE queue.
            usable = max(n_kt - 2, 1)
            quota_u = len(urgq) / usable
            quota_n = len(nxtq) / max(rem_batch[0], usable)
            acc_u = quota_u
            acc_n = quota_n

            pts = {}

            def scores(kti):
                k0 = kti * 128
                co = max(0, k0 - q0)
                sc = scp.tile([128, 2 * QCH], F32, tag="sc", name="sc")
                for h in (0, 1):
                    nc.tensor.matmul(
                        sc[:, h * QCH + co:(h + 1) * QCH],
                        kt_sb[h * 64:(h + 1) * 64,
                                  base + k0:base + k0 + 128],
                        qt_sb[h * 64:(h + 1) * 64,
                                  base + q0 + co:base + q0 + QCH],
                        start=True, stop=True)
                pt = ptp.tile([128, 2 * QCH], MMDT, tag="pt", name="pt")
                sc3 = sc.rearrange("p (h q) -> p h q", h=2)[:, :, co:QCH]
                pt3 = pt.rearrange("p (h q) -> p h q", h=2)[:, :, co:QCH]
                nc.scalar.activation(pt3, sc3, EXP)
                if k0 >= q0:
                    st = pt.rearrange("p (h q) -> p h q", h=2)[
                        :, :, co:co + 128]
                    nc.vector.tensor_tensor(
                        st, st,
                        mask_sb[:, None, :].to_broadcast((128, 2, 128)),
                        MULT)
                pts[kti] = (pt, co)

            def attnv(kti):
                pt, co = pts.pop(kti)
                nc.tensor.matmul(
                    att0[:, co:QCH],
                    v_sb[:, vbase + kti, 0:65],
                    pt[:, co:QCH],
                    start=(kti == 0), stop=(kti == n_kt - 1))
                nc.tensor.matmul(
                    att1[:, co:QCH],
                    v_sb[:, vbase + kti, 65:130],
                    pt[:, QCH + co:2 * QCH],
                    start=(kti == 0), stop=(kti == n_kt - 1))

            scores(0)
            if prev_finish is not None:
                # previous chunk's last attn@V + att evacuation, emitted
                # AFTER this chunk's first scores so the PE can start on
                # them while the previous exp drains (cross-boundary
                # software pipelining).
                prev_finish()
            for kti in range(n_kt):
                if kti + 1 < n_kt:
                    scores(kti + 1)
                if kti < usable or n_kt <= 2:
                    n = int(acc_u)
                    acc_u -= n
                    drain(urgq, n)
                    acc_u += quota_u
                    n = int(acc_n)
                    acc_n -= n
                    drain(nxtq, n)
                    acc_n += quota_n
                    # normq/outq grow mid-chunk (drained ub units feed
                    # outq), so pace them adaptively per k-tile.
                    rem_here = max(usable - kti, 1)
                    n = -(-len(normq) // rem_here)
                    drain(normq, n)
                    backlog = max(0, len(outq) - reserve[0])
                    n = -(-backlog // min(rem_here + rem_total[0], 24))
                    drain(outq, n)
                if kti < n_kt - 1:
                    attnv(kti)

            # The last attn@V + PSUM evacuation + denominator kick-off are
            # deferred into the next chunk via the finish closure.
            cols = slice(base + q0, base + q0 + QCH)
            au = rcp.tile([65, 2, QCH], F32, tag="au")
            au0 = au[:, 0, :]
            au1 = au[:, 1, :]
            NI = QCH // 128
            box = []

            def finish():
                attnv(n_kt - 1)
                nc.scalar.copy(au0, att0[:])
                nc.vector.tensor_copy(au1, att1[:])
                # reciprocal is ~8 cyc/elem/lane on 1-2 partitions.
                # Bounce through DRAM to spread the 2x512 denominators
                # over 128 partitions (recip there is ~130ns), broadcast
                # back with a stride-0 DRAM AP.
                d_dn = drp.tile([2, QCH], F32, tag="ddn", name="ddn")
                nc.gpsimd.dma_start(d_dn[:, :], au[64:65, :, :])
                sp = rcp.tile([128, 2, NI], F32, tag="sp", name="sp")
                nc.gpsimd.dma_start(
                    sp[:], d_dn.rearrange("h (p i) -> p h i", p=128))
                box.append(sp)
            bcs = []

            def unit_a():
                # run 1 chunk later: sp has long since landed, so the
                # reciprocal never stalls the DVE FIFO.
                sp = box[0]
                rcs = rcp.tile([128, 2, NI], F32, tag="rcs", name="rcs")
                nc.vector.reciprocal(rcs[:], sp[:])
                d_rc = drp.tile([2, QCH], F32, tag="drc", name="drc")
                nc.gpsimd.dma_start(
                    d_rc.rearrange("h (p i) -> p h i", p=128), rcs[:])
                bc0 = rcp.tile([64, QCH], F32, tag="bc0", name="bc0")
                bc1 = rcp.tile([64, QCH], F32, tag="bc1", name="bc1")
                nc.gpsimd.dma_start(
                    bc0[:], bass.AP(tensor=d_rc.tensor, offset=d_rc.offset,
                                    ap=[[0, 64], [1, QCH]]))
                nc.gpsimd.dma_start(
                    bc1[:], bass.AP(tensor=d_rc.tensor,
                                    offset=d_rc.offset + QCH,
                                    ap=[[0, 64], [1, QCH]]))
                bcs.extend((bc0, bc1))

            def unit_b():
                # run 2 chunks later: broadcasts are resident, normalize
                # and the head-1 partition move go straight through.
                bc0, bc1 = bcs
                nc.vector.tensor_tensor(
                    a_sb[0:64, cols], au0[0:64, :], bc0[:], MULT)
                a1_t = rcp.tile([64, QCH], MMDT, tag="a1", name="a1")
                nc.vector.tensor_tensor(
                    a1_t[:], au1[0:64, :], bc1[:], MULT)
                nc.sync.dma_start(a_sb[64:128, cols], a1_t[:])

            return {"t0": base + q0, "finish": finish, "ua": unit_a,
                    "ub": unit_b, "ua_staged": False}

        # ---------------- main schedule ---------------------------------
        # Prologue: project batch 0's first token chunk only; the rest of
        # batch 0's chunks (and every later batch's) ride as filler.
        CPB = S // TCH             # proj chunks per batch
        assert CPB == NQC
        for u in proj_chunk_units(0):
            u()

        pend = deque()          # chunk records, oldest first

        def mk_ub_unit(e):
            # when the deferred normalize drains, its chunk's output
            # projection becomes eligible (Tile derives dependencies from
            # emission order, so outproj may only be emitted after ub).
            def unit():
                e["ub"]()
                outq.extend(
                    outproj_units(e["t0"], tail=(cur_b[0] == B - 1)))
            return unit

        def age_pending(flush=False):
            # stage deferred normalize work: unit_a one chunk after its
            # attention, unit_b (which then queues the outproj) two after.
            if len(pend) >= 2 and not pend[-2]["ua_staged"]:
                normq.append(pend[-2]["ua"])
                pend[-2]["ua_staged"] = True
            if len(pend) >= 3 or (flush and pend):
                e = pend.popleft()
                if not e["ua_staged"]:
                    normq.append(e["ua"])
                    e["ua_staged"] = True
                normq.append(mk_ub_unit(e))

        RESERVE = (48, 48, 24, 0)
        prev_finish = None
        for b in range(B):
            cur_b[0] = b
            reserve[0] = RESERVE[min(b, len(RESERVE) - 1)]
            rem_batch[0] = sum(
                max((i * QCH + QCH) // 128 - 2, 1) for i in range(NQC))
            for i in range(NQC):
                qc = i              # ascending: qc i only needs chunks <= i
                # Tile derives dependencies from EMISSION order: any unit
                # whose output this attention chunk reads must be emitted
                # before it. Force-drain leftovers past their deadline.
                if i == 0:
                    drain(nxtq, len(nxtq))     # batch b's proj complete
                if b == 0:
                    drain(urgq, len(urgq))     # chunk qc of b0 complete
                if b == 0 and i + 1 < CPB:
                    urgq.extend(proj_chunk_units(i + 1))
                if b + 1 < B:
                    nxtq.extend(proj_chunk_units((b + 1) * CPB + i))
                if b == B - 1 and i == NQC - 1:
                    # stage everything pending so it drains inside the
                    # final (largest) chunk rather than serially after it
                    reserve[0] = 0
                    while pend:
                        age_pending(flush=True)
                r = attention_chunk(b, qc, prev_finish)
                prev_finish = r["finish"]
                pend.append(r)
                rem_batch[0] -= max((i * QCH + QCH) // 128 - 2, 1)
                rem_total[0] -= max((i * QCH + QCH) // 128 - 2, 1)
                age_pending()
        # tail: last chunk's deferred attn@V/evac, then flush everything
        prev_finish()
        while pend:
            age_pending(flush=True)
        drain(urgq, len(urgq))
        drain(nxtq, len(nxtq))
        while normq or outq:
            drain(normq, len(normq))
            drain(outq, len(outq))

    nc.compile()
    return nc


def prep_inputs(in_features, weight_q, weight_k, weight_v, weight_o, cfg: Cfg,
                n_cores=8):
    """Host-side shard/layout prep. Returns per-core input dicts."""
    B, S, D, T, KT = cfg.B, cfg.S, cfg.D, cfg.T, cfg.KT
    if cfg.mm_dt == "bf16":
        import ml_dtypes
        mmnp = ml_dtypes.bfloat16
    else:
        mmnp = np.float32
    x = np.asarray(in_features, dtype=np.float32).reshape(T, D)
    # xT[p, kt, t] = x[t, kt*128 + p]
    xT = np.ascontiguousarray(
        x.T.reshape(KT, 128, T).transpose(1, 0, 2))
    mask = np.triu(np.ones((128, 128), dtype=np.float32))
    wq = np.asarray(weight_q, dtype=np.float32) * (1.0 / np.sqrt(cfg.HD))
    wk = np.asarray(weight_k, dtype=np.float32)
    wv = np.asarray(weight_v, dtype=np.float32)
    wo = np.asarray(weight_o, dtype=np.float32)

    def wslice(w, c):
        # [128, KT, 128]: ws[p, kt, m] = w[c*128 + m, kt*128 + p]
        ws = w[c * 128:(c + 1) * 128, :]                  # [128, D]
        return np.ascontiguousarray(
            ws.T.reshape(KT, 128, 128).transpose(1, 0, 2))

    xT = xT.astype(mmnp)
    in_maps = []
    for c in range(n_cores):
        in_maps.append({
            "xT": xT,
            "wq": wslice(wq, c).astype(mmnp),
            "wk": wslice(wk, c).astype(mmnp),
            "wv": wslice(wv, c).astype(mmnp),
            "wo": np.ascontiguousarray(
                wo[:, c * 128:(c + 1) * 128].T).astype(mmnp),
            "mask": mask.astype(mmnp),
            "ident": np.eye(128, dtype=mmnp),
        })
    return in_maps


_CACHE = {}


def _get_program(cfg: Cfg):
    key = (cfg.B, cfg.S, cfg.D, cfg.TCH, cfg.QCH, cfg.mm_dt)
    if key not in _CACHE:
        _CACHE[key] = build_program(cfg)
    return _CACHE[key]


def run(inputs, cfg: Cfg, trace=False, trace_kwargs=None):
    import time
    from concourse.bass_utils import run_bass_kernel_spmd
    nc = _get_program(cfg)
    in_maps = prep_inputs(**inputs, cfg=cfg)
    last = None
    for attempt in range(3):
        try:
            res = run_bass_kernel_spmd(
                nc, in_maps, core_ids=list(range(8)), trace=trace,
                **(trace_kwargs or {}))
            break
        except Exception as e:  # transient NRT device wedges happen
            last = e
            time.sleep(10)
    else:
        raise last
    parts = [np.asarray(r["out_p"], dtype=np.float32) for r in res.results]
    out = np.sum(np.stack(parts, 0), axis=0)
    return out.astype(np.float32).reshape(cfg.B, cfg.S, cfg.D), res


def kernel(in_features, weight_q, weight_k, weight_v, weight_o):
    cfg = Cfg()
    out, _ = run(dict(in_features=in_features, weight_q=weight_q,
                      weight_k=weight_k, weight_v=weight_v,
                      weight_o=weight_o), cfg)
    return out


# revision 28
# speedup vs baseline: 1.0276x; 1.0276x over previous
"""Causal multi-head self-attention on 8 Trainium2 NeuronCores.

Sharding: tensor-parallel over heads. Each of the 8 cores owns 2 heads
(128 of the 1024 qkv dims). Per core:
  - QT/KT = (x @ Wq_c^T)^T etc. computed in transposed-activation layout
    [128 dims, 8192 tokens] (Wq pre-scaled by 1/sqrt(hd) on host).
  - V transposed back to natural [tokens, dims] via PE transpose, with a
    ones-column appended so the attn@V matmul also produces the softmax
    denominator (softmax computed without max-subtraction: scores are
    O(+-10) so exp() is safe in fp32).
  - scores^T = K Q^T per (batch, head), causal via per-tile widths + one
    128x128 staircase mask on the diagonal tiles. The two heads' score
    matmuls run concurrently as row-tiled PE pairs (contraction 64 at
    base partitions 0/64 -> tile_position auto-derives).
  - out_partial = A_c^T @ Wo_c^T written per core; host sums the 8
    partials (the row-parallel all-reduce done on host).

All matmul operands are bf16; accumulation and softmax denominators
stay fp32 in PSUM. Output partials are written bf16, summed fp32 on
host.

Scheduling: the attention inner loop is ScalarE-bound (one exp per
128-key tile, ~1.15us each), so the Q/K/V projections of batch b+1 and
the output projections of the previous q-chunk are sliced into small
"filler" units and interleaved into batch b's attention stream at
k-tile granularity. Scores are software-pipelined one k-tile ahead of
attn@V so a pending exp never head-of-line-blocks the next score
matmul in the PE queue. This keeps the PE dense (HAM stays at full
clock) and hides the projection phases under the exp stream.
"""

import numpy as np
from collections import deque
from contextlib import ExitStack

import concourse.bass as bass
import concourse.mybir as mybir
import concourse.tile as tile
from concourse import bacc

F32 = mybir.dt.float32
F32R = mybir.dt.float32r
BF16 = mybir.dt.bfloat16
EXP = mybir.ActivationFunctionType.Exp
MULT = mybir.AluOpType.mult


class Cfg:
    def __init__(self, B=4, S=2048, D=1024, TCH=512, QCH=512, mm_dt="bf16"):
        self.B, self.S, self.D = B, S, D
        self.T = B * S
        self.KT = D // 128          # contraction tiles for projections
        self.TCH = TCH              # token chunk for projections
        self.QCH = QCH              # query chunk for attention
        self.NQC = S // QCH         # q chunks per batch
        self.HD = 64
        self.mm_dt = mm_dt
        assert S % QCH == 0 and QCH % 128 == 0 and self.T % TCH == 0


def _mmdt(cfg):
    return {"f32r": F32R, "bf16": BF16, "f32": F32}[cfg.mm_dt]


def build_program(cfg: Cfg):
    """Build the SPMD single-core Bass program (same program all cores)."""
    nc = bacc.Bacc("TRN2", target_bir_lowering=False, debug=False)
    B, S, D, T, KT = cfg.B, cfg.S, cfg.D, cfg.T, cfg.KT
    TCH, QCH, NQC = cfg.TCH, cfg.QCH, cfg.NQC
    NVT = T // 128                 # number of 128-token V tiles
    MMDT = _mmdt(cfg)
    OC = min(512, D)

    xT_d = nc.dram_tensor("xT", [128, KT, T], MMDT, kind="ExternalInput")
    wq_d = nc.dram_tensor("wq", [128, KT, 128], MMDT, kind="ExternalInput")
    wk_d = nc.dram_tensor("wk", [128, KT, 128], MMDT, kind="ExternalInput")
    wv_d = nc.dram_tensor("wv", [128, KT, 128], MMDT, kind="ExternalInput")
    wo_d = nc.dram_tensor("wo", [128, D], MMDT, kind="ExternalInput")
    mask_d = nc.dram_tensor("mask", [128, 128], MMDT, kind="ExternalInput")
    ident_d = nc.dram_tensor("ident", [128, 128], MMDT, kind="ExternalInput")
    out_d = nc.dram_tensor("out_p", [T, D], MMDT, kind="ExternalOutput")
    out_r = out_d.rearrange("(n p) o -> p n o", p=128)   # [128, NVT, D]

    with tile.TileContext(nc) as tc, ExitStack() as ctx:
        persist = ctx.enter_context(tc.tile_pool(name="persist", bufs=1))
        xp = ctx.enter_context(tc.tile_pool(name="xp", bufs=4))
        vtp = ctx.enter_context(tc.tile_pool(name="vtp", bufs=2))
        ptp = ctx.enter_context(tc.tile_pool(name="ptp", bufs=3))
        rcp = ctx.enter_context(tc.tile_pool(name="rcp", bufs=6))
        op = ctx.enter_context(tc.tile_pool(name="op", bufs=3))
        drp = ctx.enter_context(tc.tile_pool(name="drp", bufs=6, space="DRAM"))
        # PSUM budget (8 banks): scores 2x[128,1024]f32 = 4, att 2x1 = 2,
        # shared proj/transpose/outproj pool 2x1 = 2.
        scp = ctx.enter_context(tc.tile_pool(name="scp", bufs=2, space="PSUM"))
        attp = ctx.enter_context(
            tc.tile_pool(name="attp", bufs=2, space="PSUM"))
        mp = ctx.enter_context(tc.tile_pool(name="mp", bufs=2, space="PSUM"))

        qt_sb = persist.tile([128, T], MMDT, tag="qt")
        kt_sb = persist.tile([128, T], MMDT, tag="kt")
        a_sb = persist.tile([128, T], MMDT, tag="a")
        # V natural layout, one ones-column per head so each attn@V matmul
        # also emits the softmax denominator in its last output row:
        #   cols 0:64 = head0 dims, col 64 = 1.0,
        #   cols 65:129 = head1 dims, col 129 = 1.0
        v_sb = persist.tile([128, NVT, 130], MMDT, tag="v")
        wq_sb = persist.tile([128, KT, 128], MMDT, tag="wq")
        wk_sb = persist.tile([128, KT, 128], MMDT, tag="wk")
        wv_sb = persist.tile([128, KT, 128], MMDT, tag="wv")
        wo_sb = persist.tile([128, D], MMDT, tag="wo")
        mask_sb = persist.tile([128, 128], MMDT, tag="mask")
        ident = persist.tile([128, 128], MMDT, tag="ident")
        ones128 = persist.tile([128, 1], MMDT, tag="ones128")
        nc.vector.memset(ones128[:], 1.0)

        # chunk-0 activations first in the DMA queues (1MB, feeds the
        # prologue); weights are small and follow.
        x0_t = xp.tile([128, KT, TCH], MMDT, tag="x", name="x0")
        for si in range(KT):
            nc.sync.dma_start(x0_t[:, si:si + 1, :],
                              xT_d[:, si:si + 1, 0:TCH])
        nc.sync.dma_start(wq_sb[:], wq_d[:])
        nc.sync.dma_start(wk_sb[:], wk_d[:])
        nc.sync.dma_start(wv_sb[:], wv_d[:])
        nc.sync.dma_start(wo_sb[:], wo_d[:])
        nc.sync.dma_start(mask_sb[:], mask_d[:])
        nc.sync.dma_start(ident[:], ident_d[:])
        nc.vector.tensor_copy(
            v_sb[:, :, 64:65],
            ones128[:, None, :].to_broadcast((128, NVT, 1)))
        nc.vector.tensor_copy(
            v_sb[:, :, 129:130],
            ones128[:, None, :].to_broadcast((128, NVT, 1)))
        # trigger the ~2.7us exp ACT-table load during the prologue so the
        # first real exp doesn't pay it
        warm = persist.tile([128, 1], F32, tag="warm")
        nc.scalar.activation(warm[:], ones128[:], EXP)

        # ---------------- projection filler units ----------------------
        def proj_chunk_units(tci, x_pre=None):
            """Yield closures; each emits ~0.2-0.5us of PE work projecting
            token chunk tci into qt/kt/v."""
            t0 = tci * TCH
            if x_pre is not None:
                x_t = x_pre
            else:
                x_t = xp.tile([128, KT, TCH], MMDT, tag="x")
                step = KT // 2
                for si in range(2):
                    nc.sync.dma_start(
                        x_t[:, si * step:(si + 1) * step, :],
                        xT_d[:, si * step:(si + 1) * step, t0:t0 + TCH])

            for w_sb, kind in ((wq_sb, "q"), (wk_sb, "k"), (wv_sb, "v")):
                ps_box = []

                def mk_mm(kt0, w_sb=w_sb, ps_box=ps_box):
                    def unit():
                        if not ps_box:
                            ps_box.append(mp.tile([128, TCH], F32, tag="mp", name="ps"))
                        ps = ps_box[0]
                        for kt in (kt0, kt0 + 1):
                            nc.tensor.matmul(
                                ps[:], w_sb[:, kt, :], x_t[:, kt, :],
                                start=(kt == 0), stop=(kt == KT - 1))
                    return unit
                for kt0 in range(0, KT, 2):
                    yield mk_mm(kt0)

                def mk_evac(kind=kind, ps_box=ps_box):
                    def unit():
                        ps = ps_box[0]
                        if kind == "q":
                            nc.scalar.copy(qt_sb[:, t0:t0 + TCH], ps[:])
                        elif kind == "k":
                            nc.scalar.copy(kt_sb[:, t0:t0 + TCH], ps[:])
                        else:
                            vt_t = vtp.tile([128, TCH], MMDT, tag="vt", name="vt")
                            ps_box.append(vt_t)
                            nc.vector.tensor_copy(vt_t[:], ps[:])
                    return unit
                yield mk_evac()

                if kind == "v":
                    def mk_tr(j, ps_box=ps_box):
                        def unit():
                            vt_t = ps_box[1]
                            tr = mp.tile([128, 128], MMDT, tag="mp", name="tr")
                            nc.tensor.transpose(
                                tr[:], vt_t[:, j * 128:(j + 1) * 128],
                                ident[:])
                            ktg = (t0 + j * 128) // 128
                            nc.vector.tensor_copy(
                                v_sb[:, ktg, 0:64], tr[:, 0:64])
                            nc.vector.tensor_copy(
                                v_sb[:, ktg, 65:129], tr[:, 64:128])
                        return unit
                    for j in range(TCH // 128):
                        yield mk_tr(j)

        # ---------------- output-projection filler units ----------------
        def outproj_units(t0, tail=False):
            """Yield closures for the output projection of QCH tokens
            starting at t0 (one 128-token tile per unit pair). In the
            kernel tail the PSUM evacuations alternate DVE/ACT (ACT is
            idle there) to double the drain rate."""
            for ti in range(QCH // 128):
                tt = t0 // 128 + ti
                o_box = []

                def mk_oc(oc, ti=ti, tt=tt, o_box=o_box):
                    def unit():
                        if not o_box:
                            o_box.append(op.tile([128, D], MMDT, tag="osb", name="osb"))
                        o_sb = o_box[0]
                        ps = mp.tile([128, OC], F32, tag="mp", name="wops")
                        nc.tensor.matmul(
                            ps[:],
                            a_sb[:, tt * 128:(tt + 1) * 128],
                            wo_sb[:, oc * OC:(oc + 1) * OC],
                            start=True, stop=True)
                        if tail and (ti + oc) % 2:
                            nc.scalar.copy(
                                o_sb[:, oc * OC:(oc + 1) * OC], ps[:])
                        else:
                            nc.vector.tensor_copy(
                                o_sb[:, oc * OC:(oc + 1) * OC], ps[:])
                        if oc == D // OC - 1:
                            nc.sync.dma_start(out_r[:, tt, :], o_sb[:])
                    return unit
                for oc in range(D // OC):
                    yield mk_oc(oc)

        urgq = deque()          # own-batch proj: deadline = end of chunk
        nxtq = deque()          # next-batch proj: deadline = end of batch
        normq = deque()         # deferred softmax-normalize (ua/ub units)
        outq = deque()          # outproj only; fed by drained ub units
        rem_batch = [0]         # usable k-tiles left in current batch
        rem_total = [sum(max((i * QCH + QCH) // 128 - 2, 1)
                         for i in range(NQC)) * B]
        reserve = [0]           # outq units held back for late batches
        cur_b = [0]

        def drain(q, n):
            for _ in range(n):
                if not q:
                    return
                q.popleft()()

        # ---------------- attention chunk -------------------------------
        def attention_chunk(b, qc, prev_finish=None):
            """Scores + softmax + attn@V + normalization for one
            (batch, q-chunk), with scores pipelined one k-tile ahead and
            filler drained between k-tiles."""
            base = b * S
            vbase = base // 128
            q0 = qc * QCH
            n_kt = (q0 + QCH) // 128
            att0 = attp.tile([65, QCH], F32, tag="att")
            att1 = attp.tile([65, QCH], F32, tag="att")

            # pacing: urgent proj finishes within this chunk, next-batch
            # proj within the current batch, outproj/normalize within a
            # ~24-k-tile horizon (bounds the end-of-kernel backlog). The
            # last 2 k-tiles of a chunk drain nothing, so the att-PSUM
            # evacuations land at the head of the DVE queue.
            usable = max(n_kt - 2, 1)
            quota_u = len(urgq) / usable
            quota_n = len(nxtq) / max(rem_batch[0], usable)
            acc_u = quota_u
            acc_n = quota_n

            pts = {}

            def scores(kti):
                k0 = kti * 128
                co = max(0, k0 - q0)
                sc = scp.tile([128, 2 * QCH], F32, tag="sc", name="sc")
                for h in (0, 1):
                    nc.tensor.matmul(
                        sc[:, h * QCH + co:(h + 1) * QCH],
                        kt_sb[h * 64:(h + 1) * 64,
                                  base + k0:base + k0 + 128],
                        qt_sb[h * 64:(h + 1) * 64,
                                  base + q0 + co:base + q0 + QCH],
                        start=True, stop=True)
                pt = ptp.tile([128, 2 * QCH], MMDT, tag="pt", name="pt")
                sc3 = sc.rearrange("p (h q) -> p h q", h=2)[:, :, co:QCH]
                pt3 = pt.rearrange("p (h q) -> p h q", h=2)[:, :, co:QCH]
                nc.scalar.activation(pt3, sc3, EXP)
                if k0 >= q0:
                    st = pt.rearrange("p (h q) -> p h q", h=2)[
                        :, :, co:co + 128]
                    nc.vector.tensor_tensor(
                        st, st,
                        mask_sb[:, None, :].to_broadcast((128, 2, 128)),
                        MULT)
                pts[kti] = (pt, co)

            def attnv(kti):
                pt, co = pts.pop(kti)
                nc.tensor.matmul(
                    att0[:, co:QCH],
                    v_sb[:, vbase + kti, 0:65],
                    pt[:, co:QCH],
                    start=(kti == 0), stop=(kti == n_kt - 1))
                nc.tensor.matmul(
                    att1[:, co:QCH],
                    v_sb[:, vbase + kti, 65:130],
                    pt[:, QCH + co:2 * QCH],
                    start=(kti == 0), stop=(kti == n_kt - 1))

            scores(0)
            if prev_finish is not None:
                # previous chunk's last attn@V + att evacuation, emitted
                # AFTER this chunk's first scores so the PE can start on
                # them while the previous exp drains (cross-boundary
                # software pipelining).
                prev_finish()
            for kti in range(n_kt):
                if kti + 1 < n_kt:
                    scores(kti + 1)
                if kti < usable or n_kt <= 2:
                    n = int(acc_u)
                    acc_u -= n
                    drain(urgq, n)
                    acc_u += quota_u
                    n = int(acc_n)
                    acc_n -= n
                    drain(nxtq, n)
                    acc_n += quota_n
                    # normq/outq grow mid-chunk (drained ub units feed
                    # outq), so pace them adaptively per k-tile.
                    rem_here = max(usable - kti, 1)
                    n = -(-len(normq) // rem_here)
                    drain(normq, n)
                    backlog = max(0, len(outq) - reserve[0])
                    n = -(-backlog // min(rem_here + rem_total[0], 24))
                    drain(outq, n)
                if kti < n_kt - 1:
                    attnv(kti)

            # The last attn@V + PSUM evacuation + denominator kick-off are
            # deferred into the next chunk via the finish closure.
            cols = slice(base + q0, base + q0 + QCH)
            au = rcp.tile([65, 2, QCH], F32, tag="au")
            au0 = au[:, 0, :]
            au1 = au[:, 1, :]
            NI = QCH // 128
            box = []

            def finish():
                attnv(n_kt - 1)
                nc.scalar.copy(au0, att0[:])
                nc.vector.tensor_copy(au1, att1[:])
                # reciprocal is ~8 cyc/elem/lane on 1-2 partitions.
                # Bounce through DRAM to spread the 2x512 denominators
                # over 128 partitions (recip there is ~130ns), broadcast
                # back with a stride-0 DRAM AP.
                d_dn = drp.tile([2, QCH], F32, tag="ddn", name="ddn")
                nc.gpsimd.dma_start(d_dn[:, :], au[64:65, :, :])
                sp = rcp.tile([128, 2, NI], F32, tag="sp", name="sp")
                nc.gpsimd.dma_start(
                    sp[:], d_dn.rearrange("h (p i) -> p h i", p=128))
                box.append(sp)
            bcs = []

            def unit_a():
                # run 1 chunk later: sp has long since landed, so the
                # reciprocal never stalls the DVE FIFO.
                sp = box[0]
                rcs = rcp.tile([128, 2, NI], F32, tag="rcs", name="rcs")
                nc.vector.reciprocal(rcs[:], sp[:])
                d_rc = drp.tile([2, QCH], F32, tag="drc", name="drc")
                nc.gpsimd.dma_start(
                    d_rc.rearrange("h (p i) -> p h i", p=128), rcs[:])
                bc0 = rcp.tile([64, QCH], F32, tag="bc0", name="bc0")
                bc1 = rcp.tile([64, QCH], F32, tag="bc1", name="bc1")
                nc.gpsimd.dma_start(
                    bc0[:], bass.AP(tensor=d_rc.tensor, offset=d_rc.offset,
                                    ap=[[0, 64], [1, QCH]]))
                nc.gpsimd.dma_start(
                    bc1[:], bass.AP(tensor=d_rc.tensor,
                                    offset=d_rc.offset + QCH,
                                    ap=[[0, 64], [1, QCH]]))
                bcs.extend((bc0, bc1))

            def unit_b():
                # run 2 chunks later: broadcasts are resident, normalize
                # and the head-1 partition move go straight through.
                bc0, bc1 = bcs
                nc.vector.tensor_tensor(
                    a_sb[0:64, cols], au0[0:64, :], bc0[:], MULT)
                a1_t = rcp.tile([64, QCH], MMDT, tag="a1", name="a1")
                nc.vector.tensor_tensor(
                    a1_t[:], au1[0:64, :], bc1[:], MULT)
                nc.sync.dma_start(a_sb[64:128, cols], a1_t[:])

            return {"t0": base + q0, "finish": finish, "ua": unit_a,
                    "ub": unit_b, "ua_staged": False}

        # ---------------- main schedule ---------------------------------
        # Prologue: project batch 0's first token chunk only; the rest of
        # batch 0's chunks (and every later batch's) ride as filler.
        CPB = S // TCH             # proj chunks per batch
        assert CPB == NQC
        for u in proj_chunk_units(0, x_pre=x0_t):
            u()

        pend = deque()          # chunk records, oldest first

        def mk_ub_unit(e):
            # when the deferred normalize drains, its chunk's output
            # projection becomes eligible (Tile derives dependencies from
            # emission order, so outproj may only be emitted after ub).
            def unit():
                e["ub"]()
                outq.extend(
                    outproj_units(e["t0"], tail=(cur_b[0] == B - 1)))
            return unit

        def age_pending(flush=False):
            # stage deferred normalize work: unit_a one chunk after its
            # attention, unit_b (which then queues the outproj) two after.
            if len(pend) >= 2 and not pend[-2]["ua_staged"]:
                normq.append(pend[-2]["ua"])
                pend[-2]["ua_staged"] = True
            if len(pend) >= 3 or (flush and pend):
                e = pend.popleft()
                if not e["ua_staged"]:
                    normq.append(e["ua"])
                    e["ua_staged"] = True
                normq.append(mk_ub_unit(e))

        RESERVE = (48, 48, 24, 0)
        prev_finish = None
        for b in range(B):
            cur_b[0] = b
            reserve[0] = RESERVE[min(b, len(RESERVE) - 1)]
            rem_batch[0] = sum(
                max((i * QCH + QCH) // 128 - 2, 1) for i in range(NQC))
            for i in range(NQC):
                qc = i              # ascending: qc i only needs chunks <= i
                # Tile derives dependencies from EMISSION order: any unit
                # whose output this attention chunk reads must be emitted
                # before it. Force-drain leftovers past their deadline.
                if i == 0:
                    drain(nxtq, len(nxtq))     # batch b's proj complete
                if b == 0:
                    drain(urgq, len(urgq))     # chunk qc of b0 complete
                if b == 0 and i + 1 < CPB:
                    urgq.extend(proj_chunk_units(i + 1))
                if b + 1 < B:
                    nxtq.extend(proj_chunk_units((b + 1) * CPB + i))
                if b == B - 1 and i == NQC - 1:
                    # stage everything pending so it drains inside the
                    # final (largest) chunk rather than serially after it
                    reserve[0] = 0
                    while pend:
                        age_pending(flush=True)
                r = attention_chunk(b, qc, prev_finish)
                prev_finish = r["finish"]
                pend.append(r)
                rem_batch[0] -= max((i * QCH + QCH) // 128 - 2, 1)
                rem_total[0] -= max((i * QCH + QCH) // 128 - 2, 1)
                age_pending()
        # tail: last chunk's deferred attn@V/evac, then flush everything
        prev_finish()
        while pend:
            age_pending(flush=True)
        drain(urgq, len(urgq))
        drain(nxtq, len(nxtq))
        while normq or outq:
            drain(normq, len(normq))
            drain(outq, len(outq))

    nc.compile()
    return nc


def prep_inputs(in_features, weight_q, weight_k, weight_v, weight_o, cfg: Cfg,
                n_cores=8):
    """Host-side shard/layout prep. Returns per-core input dicts."""
    B, S, D, T, KT = cfg.B, cfg.S, cfg.D, cfg.T, cfg.KT
    if cfg.mm_dt == "bf16":
        import ml_dtypes
        mmnp = ml_dtypes.bfloat16
    else:
        mmnp = np.float32
    x = np.asarray(in_features, dtype=np.float32).reshape(T, D)
    # xT[p, kt, t] = x[t, kt*128 + p]
    xT = np.ascontiguousarray(
        x.T.reshape(KT, 128, T).transpose(1, 0, 2))
    mask = np.triu(np.ones((128, 128), dtype=np.float32))
    wq = np.asarray(weight_q, dtype=np.float32) * (1.0 / np.sqrt(cfg.HD))
    wk = np.asarray(weight_k, dtype=np.float32)
    wv = np.asarray(weight_v, dtype=np.float32)
    wo = np.asarray(weight_o, dtype=np.float32)

    def wslice(w, c):
        # [128, KT, 128]: ws[p, kt, m] = w[c*128 + m, kt*128 + p]
        ws = w[c * 128:(c + 1) * 128, :]                  # [128, D]
        return np.ascontiguousarray(
            ws.T.reshape(KT, 128, 128).transpose(1, 0, 2))

    xT = xT.astype(mmnp)
    in_maps = []
    for c in range(n_cores):
        in_maps.append({
            "xT": xT,
            "wq": wslice(wq, c).astype(mmnp),
            "wk": wslice(wk, c).astype(mmnp),
            "wv": wslice(wv, c).astype(mmnp),
            "wo": np.ascontiguousarray(
                wo[:, c * 128:(c + 1) * 128].T).astype(mmnp),
            "mask": mask.astype(mmnp),
            "ident": np.eye(128, dtype=mmnp),
        })
    return in_maps


_CACHE = {}


def _get_program(cfg: Cfg):
    key = (cfg.B, cfg.S, cfg.D, cfg.TCH, cfg.QCH, cfg.mm_dt)
    if key not in _CACHE:
        _CACHE[key] = build_program(cfg)
    return _CACHE[key]


def run(inputs, cfg: Cfg, trace=False, trace_kwargs=None):
    import time
    from concourse.bass_utils import run_bass_kernel_spmd
    nc = _get_program(cfg)
    in_maps = prep_inputs(**inputs, cfg=cfg)
    last = None
    for attempt in range(3):
        try:
            res = run_bass_kernel_spmd(
                nc, in_maps, core_ids=list(range(8)), trace=trace,
                **(trace_kwargs or {}))
            break
        except Exception as e:  # transient NRT device wedges happen
            last = e
            time.sleep(10)
    else:
        raise last
    parts = [np.asarray(r["out_p"], dtype=np.float32) for r in res.results]
    out = np.sum(np.stack(parts, 0), axis=0)
    return out.astype(np.float32).reshape(cfg.B, cfg.S, cfg.D), res


def kernel(in_features, weight_q, weight_k, weight_v, weight_o):
    cfg = Cfg()
    out, _ = run(dict(in_features=in_features, weight_q=weight_q,
                      weight_k=weight_k, weight_v=weight_v,
                      weight_o=weight_o), cfg)
    return out


# revision 29
# speedup vs baseline: 1.0612x; 1.0327x over previous
"""Causal multi-head self-attention on 8 Trainium2 NeuronCores.

Sharding: tensor-parallel over heads. Each of the 8 cores owns 2 heads
(128 of the 1024 qkv dims). Per core:
  - QT/KT = (x @ Wq_c^T)^T etc. computed in transposed-activation layout
    [128 dims, 8192 tokens] (Wq pre-scaled by 1/sqrt(hd) on host).
  - V transposed back to natural [tokens, dims] via PE transpose, with a
    ones-column appended so the attn@V matmul also produces the softmax
    denominator (softmax computed without max-subtraction: scores are
    O(+-10) so exp() is safe in fp32).
  - scores^T = K Q^T per (batch, head), causal via per-tile widths + one
    128x128 staircase mask on the diagonal tiles. The two heads' score
    matmuls run concurrently as row-tiled PE pairs (contraction 64 at
    base partitions 0/64 -> tile_position auto-derives).
  - out_partial = A_c^T @ Wo_c^T written per core; host sums the 8
    partials (the row-parallel all-reduce done on host).

All matmul operands are bf16; accumulation and softmax denominators
stay fp32 in PSUM. Output partials are written bf16, summed fp32 on
host.

Scheduling: the attention inner loop is ScalarE-bound (one exp per
128-key tile, ~1.15us each), so the Q/K/V projections of batch b+1 and
the output projections of the previous q-chunk are sliced into small
"filler" units and interleaved into batch b's attention stream at
k-tile granularity. Scores are software-pipelined one k-tile ahead of
attn@V so a pending exp never head-of-line-blocks the next score
matmul in the PE queue. This keeps the PE dense (HAM stays at full
clock) and hides the projection phases under the exp stream.
"""

import numpy as np
from collections import deque
from contextlib import ExitStack

import concourse.bass as bass
import concourse.mybir as mybir
import concourse.tile as tile
from concourse import bacc

F32 = mybir.dt.float32
F32R = mybir.dt.float32r
BF16 = mybir.dt.bfloat16
EXP = mybir.ActivationFunctionType.Exp
MULT = mybir.AluOpType.mult


class Cfg:
    def __init__(self, B=4, S=2048, D=1024, TCH=512, QCH=512, mm_dt="bf16"):
        self.B, self.S, self.D = B, S, D
        self.T = B * S
        self.KT = D // 128          # contraction tiles for projections
        self.TCH = TCH              # token chunk for projections
        self.QCH = QCH              # query chunk for attention
        self.NQC = S // QCH         # q chunks per batch
        self.HD = 64
        self.mm_dt = mm_dt
        assert S % QCH == 0 and QCH % 128 == 0 and self.T % TCH == 0


def _mmdt(cfg):
    return {"f32r": F32R, "bf16": BF16, "f32": F32}[cfg.mm_dt]


def build_program(cfg: Cfg):
    """Build the SPMD single-core Bass program (same program all cores)."""
    nc = bacc.Bacc("TRN2", target_bir_lowering=False, debug=False)
    B, S, D, T, KT = cfg.B, cfg.S, cfg.D, cfg.T, cfg.KT
    TCH, QCH, NQC = cfg.TCH, cfg.QCH, cfg.NQC
    NVT = T // 128                 # number of 128-token V tiles
    MMDT = _mmdt(cfg)
    OC = min(512, D)

    xT_d = nc.dram_tensor("xT", [128, KT, T], MMDT, kind="ExternalInput")
    wq_d = nc.dram_tensor("wq", [128, KT, 128], MMDT, kind="ExternalInput")
    wk_d = nc.dram_tensor("wk", [128, KT, 128], MMDT, kind="ExternalInput")
    wv_d = nc.dram_tensor("wv", [128, KT, 128], MMDT, kind="ExternalInput")
    wo_d = nc.dram_tensor("wo", [128, D], MMDT, kind="ExternalInput")
    mask_d = nc.dram_tensor("mask", [128, 128], MMDT, kind="ExternalInput")
    ident_d = nc.dram_tensor("ident", [128, 128], MMDT, kind="ExternalInput")
    out_d = nc.dram_tensor("out_p", [T, D], MMDT, kind="ExternalOutput")
    out_r = out_d.rearrange("(n p) o -> p n o", p=128)   # [128, NVT, D]

    with tile.TileContext(nc) as tc, ExitStack() as ctx:
        persist = ctx.enter_context(tc.tile_pool(name="persist", bufs=1))
        xp = ctx.enter_context(tc.tile_pool(name="xp", bufs=4))
        vtp = ctx.enter_context(tc.tile_pool(name="vtp", bufs=2))
        ptp = ctx.enter_context(tc.tile_pool(name="ptp", bufs=3))
        rcp = ctx.enter_context(tc.tile_pool(name="rcp", bufs=6))
        op = ctx.enter_context(tc.tile_pool(name="op", bufs=3))
        drp = ctx.enter_context(tc.tile_pool(name="drp", bufs=6, space="DRAM"))
        # PSUM budget (8 banks): scores 2x[128,1024]f32 = 4, att 2x1 = 2,
        # shared proj/transpose/outproj pool 2x1 = 2.
        scp = ctx.enter_context(tc.tile_pool(name="scp", bufs=2, space="PSUM"))
        attp = ctx.enter_context(
            tc.tile_pool(name="attp", bufs=2, space="PSUM"))
        mp = ctx.enter_context(tc.tile_pool(name="mp", bufs=2, space="PSUM"))

        qt_sb = persist.tile([128, T], MMDT, tag="qt")
        kt_sb = persist.tile([128, T], MMDT, tag="kt")
        a_sb = persist.tile([128, T], MMDT, tag="a")
        # V natural layout, one ones-column per head so each attn@V matmul
        # also emits the softmax denominator in its last output row:
        #   cols 0:64 = head0 dims, col 64 = 1.0,
        #   cols 65:129 = head1 dims, col 129 = 1.0
        v_sb = persist.tile([128, NVT, 130], MMDT, tag="v")
        wq_sb = persist.tile([128, KT, 128], MMDT, tag="wq")
        wk_sb = persist.tile([128, KT, 128], MMDT, tag="wk")
        wv_sb = persist.tile([128, KT, 128], MMDT, tag="wv")
        wo_sb = persist.tile([128, D], MMDT, tag="wo")
        mask_sb = persist.tile([128, 128], MMDT, tag="mask")
        ident = persist.tile([128, 128], MMDT, tag="ident")
        ones128 = persist.tile([128, 1], MMDT, tag="ones128")
        nc.vector.memset(ones128[:], 1.0)

        # chunk-0 activations first in the DMA queues (1MB, feeds the
        # prologue); weights are small and follow.
        x0_t = xp.tile([128, KT, TCH], MMDT, tag="x", name="x0")
        for si in range(KT):
            nc.sync.dma_start(x0_t[:, si:si + 1, :],
                              xT_d[:, si:si + 1, 0:TCH])
        nc.sync.dma_start(wq_sb[:], wq_d[:])
        nc.sync.dma_start(wk_sb[:], wk_d[:])
        nc.sync.dma_start(wv_sb[:], wv_d[:])
        nc.sync.dma_start(wo_sb[:], wo_d[:])
        nc.sync.dma_start(mask_sb[:], mask_d[:])
        nc.sync.dma_start(ident[:], ident_d[:])
        nc.vector.tensor_copy(
            v_sb[:, :, 64:65],
            ones128[:, None, :].to_broadcast((128, NVT, 1)))
        nc.vector.tensor_copy(
            v_sb[:, :, 129:130],
            ones128[:, None, :].to_broadcast((128, NVT, 1)))
        # trigger the ~2.7us exp ACT-table load during the prologue so the
        # first real exp doesn't pay it
        warm = persist.tile([128, 1], F32, tag="warm")
        nc.scalar.activation(warm[:], ones128[:], EXP)

        # ---------------- projection filler units ----------------------
        def proj_chunk_units(tci, x_pre=None):
            """Yield closures; each emits ~0.2-0.5us of PE work projecting
            token chunk tci into qt/kt/v."""
            t0 = tci * TCH
            if x_pre is not None:
                x_t = x_pre
            else:
                x_t = xp.tile([128, KT, TCH], MMDT, tag="x")
                step = KT // 2
                for si in range(2):
                    nc.sync.dma_start(
                        x_t[:, si * step:(si + 1) * step, :],
                        xT_d[:, si * step:(si + 1) * step, t0:t0 + TCH])

            for w_sb, kind in ((wq_sb, "q"), (wk_sb, "k"), (wv_sb, "v")):
                ps_box = []

                def mk_mm(kt0, w_sb=w_sb, ps_box=ps_box):
                    def unit():
                        if not ps_box:
                            ps_box.append(mp.tile([128, TCH], F32, tag="mp", name="ps"))
                        ps = ps_box[0]
                        for kt in (kt0, kt0 + 1):
                            nc.tensor.matmul(
                                ps[:], w_sb[:, kt, :], x_t[:, kt, :],
                                start=(kt == 0), stop=(kt == KT - 1))
                    return unit
                for kt0 in range(0, KT, 2):
                    yield mk_mm(kt0)

                def mk_evac(kind=kind, ps_box=ps_box):
                    def unit():
                        ps = ps_box[0]
                        if kind == "q":
                            nc.scalar.copy(qt_sb[:, t0:t0 + TCH], ps[:])
                        elif kind == "k":
                            nc.scalar.copy(kt_sb[:, t0:t0 + TCH], ps[:])
                        else:
                            vt_t = vtp.tile([128, TCH], MMDT, tag="vt", name="vt")
                            ps_box.append(vt_t)
                            nc.vector.tensor_copy(vt_t[:], ps[:])
                    return unit
                yield mk_evac()

                if kind == "v":
                    def mk_tr(j, ps_box=ps_box):
                        def unit():
                            vt_t = ps_box[1]
                            tr = mp.tile([128, 128], MMDT, tag="mp", name="tr")
                            nc.tensor.transpose(
                                tr[:], vt_t[:, j * 128:(j + 1) * 128],
                                ident[:])
                            ktg = (t0 + j * 128) // 128
                            nc.vector.tensor_copy(
                                v_sb[:, ktg, 0:64], tr[:, 0:64])
                            nc.vector.tensor_copy(
                                v_sb[:, ktg, 65:129], tr[:, 64:128])
                        return unit
                    for j in range(TCH // 128):
                        yield mk_tr(j)

        # ---------------- output-projection filler units ----------------
        def outproj_units(t0, tail=False):
            """Yield closures for the output projection of QCH tokens
            starting at t0 (one 128-token tile per unit pair). In the
            kernel tail the PSUM evacuations alternate DVE/ACT (ACT is
            idle there) to double the drain rate."""
            for ti in range(QCH // 128):
                tt = t0 // 128 + ti
                o_box = []

                def mk_oc(oc, ti=ti, tt=tt, o_box=o_box):
                    def unit():
                        if not o_box:
                            o_box.append(op.tile([128, D], MMDT, tag="osb", name="osb"))
                        o_sb = o_box[0]
                        ps = mp.tile([128, OC], F32, tag="mp", name="wops")
                        nc.tensor.matmul(
                            ps[:],
                            a_sb[:, tt * 128:(tt + 1) * 128],
                            wo_sb[:, oc * OC:(oc + 1) * OC],
                            start=True, stop=True)
                        if tail and (ti + oc) % 2:
                            nc.scalar.copy(
                                o_sb[:, oc * OC:(oc + 1) * OC], ps[:])
                        else:
                            nc.vector.tensor_copy(
                                o_sb[:, oc * OC:(oc + 1) * OC], ps[:])
                        if oc == D // OC - 1:
                            nc.sync.dma_start(out_r[:, tt, :], o_sb[:])
                    return unit
                for oc in range(D // OC):
                    yield mk_oc(oc)

        urgq = deque()          # own-batch proj: deadline = end of chunk
        nxtq = deque()          # next-batch proj: deadline = end of batch
        normq = deque()         # deferred softmax-normalize (ua/ub units)
        outq = deque()          # outproj only; fed by drained ub units
        rem_batch = [0]         # usable k-tiles left in current batch
        rem_total = [sum(max((i * QCH + QCH) // 128 - 2, 1)
                         for i in range(NQC)) * B]
        reserve = [0]           # outq units held back for late batches
        cur_b = [0]

        def drain(q, n):
            for _ in range(n):
                if not q:
                    return
                q.popleft()()

        # ---------------- attention chunk -------------------------------
        def attention_chunk(b, qc, prev_finish=None):
            """Scores + softmax + attn@V + normalization for one
            (batch, q-chunk), with scores pipelined one k-tile ahead and
            filler drained between k-tiles."""
            base = b * S
            vbase = base // 128
            q0 = qc * QCH
            n_kt = (q0 + QCH) // 128
            att0 = attp.tile([65, QCH], F32, tag="att")
            att1 = attp.tile([65, QCH], F32, tag="att")

            # pacing: urgent proj finishes within this chunk, next-batch
            # proj within the current batch, outproj/normalize within a
            # ~24-k-tile horizon (bounds the end-of-kernel backlog). The
            # last 2 k-tiles of a chunk drain nothing, so the att-PSUM
            # evacuations land at the head of the DVE queue.
            usable = max(n_kt - 2, 1)
            quota_u = len(urgq) / usable
            quota_n = len(nxtq) / max(rem_batch[0], usable)
            acc_u = quota_u
            acc_n = quota_n

            pts = {}

            def scores(kti):
                k0 = kti * 128
                co = max(0, k0 - q0)
                sc = scp.tile([128, 2 * QCH], F32, tag="sc", name="sc")
                for h in (0, 1):
                    nc.tensor.matmul(
                        sc[:, h * QCH + co:(h + 1) * QCH],
                        kt_sb[h * 64:(h + 1) * 64,
                                  base + k0:base + k0 + 128],
                        qt_sb[h * 64:(h + 1) * 64,
                                  base + q0 + co:base + q0 + QCH],
                        start=True, stop=True)
                pt = ptp.tile([128, 2 * QCH], MMDT, tag="pt", name="pt")
                sc3 = sc.rearrange("p (h q) -> p h q", h=2)[:, :, co:QCH]
                pt3 = pt.rearrange("p (h q) -> p h q", h=2)[:, :, co:QCH]
                nc.scalar.activation(pt3, sc3, EXP)
                if k0 >= q0:
                    st = pt.rearrange("p (h q) -> p h q", h=2)[
                        :, :, co:co + 128]
                    nc.vector.tensor_tensor(
                        st, st,
                        mask_sb[:, None, :].to_broadcast((128, 2, 128)),
                        MULT)
                pts[kti] = (pt, co)

            def attnv(kti):
                pt, co = pts.pop(kti)
                nc.tensor.matmul(
                    att0[:, co:QCH],
                    v_sb[:, vbase + kti, 0:65],
                    pt[:, co:QCH],
                    start=(kti == 0), stop=(kti == n_kt - 1))
                nc.tensor.matmul(
                    att1[:, co:QCH],
                    v_sb[:, vbase + kti, 65:130],
                    pt[:, QCH + co:2 * QCH],
                    start=(kti == 0), stop=(kti == n_kt - 1))

            scores(0)
            if prev_finish is not None:
                # previous chunk's last attn@V + att evacuation, emitted
                # AFTER this chunk's first scores so the PE can start on
                # them while the previous exp drains (cross-boundary
                # software pipelining).
                prev_finish()
            for kti in range(n_kt):
                if kti + 1 < n_kt:
                    scores(kti + 1)
                if kti < usable or n_kt <= 2:
                    n = int(acc_u)
                    acc_u -= n
                    drain(urgq, n)
                    acc_u += quota_u
                    n = int(acc_n)
                    acc_n -= n
                    drain(nxtq, n)
                    acc_n += quota_n
                    # normq/outq grow mid-chunk (drained ub units feed
                    # outq), so pace them adaptively per k-tile.
                    rem_here = max(usable - kti, 1)
                    n = -(-len(normq) // rem_here)
                    drain(normq, n)
                    backlog = max(0, len(outq) - reserve[0])
                    n = -(-backlog // min(rem_here + rem_total[0], 24))
                    drain(outq, n)
                if kti < n_kt - 1:
                    attnv(kti)

            # The last attn@V + PSUM evacuation + denominator kick-off are
            # deferred into the next chunk via the finish closure.
            cols = slice(base + q0, base + q0 + QCH)
            au = rcp.tile([65, 2, QCH], F32, tag="au")
            au0 = au[:, 0, :]
            au1 = au[:, 1, :]
            NI = QCH // 128
            box = []

            def finish():
                attnv(n_kt - 1)
                nc.scalar.copy(au0, att0[:])
                nc.vector.tensor_copy(au1, att1[:])
                # reciprocal is ~8 cyc/elem/lane on 1-2 partitions.
                # Bounce through DRAM to spread the 2x512 denominators
                # over 128 partitions (recip there is ~130ns), broadcast
                # back with a stride-0 DRAM AP.
                d_dn = drp.tile([2, QCH], F32, tag="ddn", name="ddn")
                nc.sync.dma_start(d_dn[:, :], au[64:65, :, :])
                sp = rcp.tile([128, 2, NI], F32, tag="sp", name="sp")
                nc.sync.dma_start(
                    sp[:], d_dn.rearrange("h (p i) -> p h i", p=128))
                box.append(sp)
            bcs = []

            def unit_a():
                # run 1 chunk later: sp has long since landed, so the
                # reciprocal never stalls the DVE FIFO.
                sp = box[0]
                rcs = rcp.tile([128, 2, NI], F32, tag="rcs", name="rcs")
                nc.vector.reciprocal(rcs[:], sp[:])
                d_rc = drp.tile([2, QCH], F32, tag="drc", name="drc")
                nc.sync.dma_start(
                    d_rc.rearrange("h (p i) -> p h i", p=128), rcs[:])
                bc0 = rcp.tile([64, QCH], F32, tag="bc0", name="bc0")
                bc1 = rcp.tile([64, QCH], F32, tag="bc1", name="bc1")
                nc.sync.dma_start(
                    bc0[:], bass.AP(tensor=d_rc.tensor, offset=d_rc.offset,
                                    ap=[[0, 64], [1, QCH]]))
                nc.sync.dma_start(
                    bc1[:], bass.AP(tensor=d_rc.tensor,
                                    offset=d_rc.offset + QCH,
                                    ap=[[0, 64], [1, QCH]]))
                bcs.extend((bc0, bc1))

            def unit_b():
                # run 2 chunks later: broadcasts are resident, normalize
                # and the head-1 partition move go straight through.
                bc0, bc1 = bcs
                nc.vector.tensor_tensor(
                    a_sb[0:64, cols], au0[0:64, :], bc0[:], MULT)
                a1_t = rcp.tile([64, QCH], MMDT, tag="a1", name="a1")
                nc.vector.tensor_tensor(
                    a1_t[:], au1[0:64, :], bc1[:], MULT)
                nc.sync.dma_start(a_sb[64:128, cols], a1_t[:])

            return {"t0": base + q0, "finish": finish, "ua": unit_a,
                    "ub": unit_b, "ua_staged": False}

        # ---------------- main schedule ---------------------------------
        # Prologue: project batch 0's first token chunk only; the rest of
        # batch 0's chunks (and every later batch's) ride as filler.
        CPB = S // TCH             # proj chunks per batch
        assert CPB == NQC
        for u in proj_chunk_units(0, x_pre=x0_t):
            u()

        pend = deque()          # chunk records, oldest first

        def mk_ub_unit(e):
            # when the deferred normalize drains, its chunk's output
            # projection becomes eligible (Tile derives dependencies from
            # emission order, so outproj may only be emitted after ub).
            def unit():
                e["ub"]()
                outq.extend(
                    outproj_units(e["t0"], tail=(cur_b[0] == B - 1)))
            return unit

        def age_pending(flush=False):
            # stage deferred normalize work: unit_a one chunk after its
            # attention, unit_b (which then queues the outproj) two after.
            if len(pend) >= 2 and not pend[-2]["ua_staged"]:
                normq.append(pend[-2]["ua"])
                pend[-2]["ua_staged"] = True
            if len(pend) >= 3 or (flush and pend):
                e = pend.popleft()
                if not e["ua_staged"]:
                    normq.append(e["ua"])
                    e["ua_staged"] = True
                normq.append(mk_ub_unit(e))

        RESERVE = (32, 24, 8, 0)
        prev_finish = None
        for b in range(B):
            cur_b[0] = b
            reserve[0] = RESERVE[min(b, len(RESERVE) - 1)]
            rem_batch[0] = sum(
                max((i * QCH + QCH) // 128 - 2, 1) for i in range(NQC))
            for i in range(NQC):
                qc = i              # ascending: qc i only needs chunks <= i
                # Tile derives dependencies from EMISSION order: any unit
                # whose output this attention chunk reads must be emitted
                # before it. Force-drain leftovers past their deadline.
                if i == 0:
                    drain(nxtq, len(nxtq))     # batch b's proj complete
                if b == 0:
                    drain(urgq, len(urgq))     # chunk qc of b0 complete
                if b == 0 and i + 1 < CPB:
                    urgq.extend(proj_chunk_units(i + 1))
                if b + 1 < B:
                    nxtq.extend(proj_chunk_units((b + 1) * CPB + i))
                if b == B - 1 and i == NQC - 1:
                    # stage everything pending so it drains inside the
                    # final (largest) chunk rather than serially after it
                    reserve[0] = 0
                    while pend:
                        age_pending(flush=True)
                r = attention_chunk(b, qc, prev_finish)
                prev_finish = r["finish"]
                pend.append(r)
                rem_batch[0] -= max((i * QCH + QCH) // 128 - 2, 1)
                rem_total[0] -= max((i * QCH + QCH) // 128 - 2, 1)
                age_pending()
        # tail: last chunk's deferred attn@V/evac, then flush everything
        prev_finish()
        while pend:
            age_pending(flush=True)
        drain(urgq, len(urgq))
        drain(nxtq, len(nxtq))
        while normq or outq:
            drain(normq, len(normq))
            drain(outq, len(outq))

    nc.compile()
    return nc


def prep_inputs(in_features, weight_q, weight_k, weight_v, weight_o, cfg: Cfg,
                n_cores=8):
    """Host-side shard/layout prep. Returns per-core input dicts."""
    B, S, D, T, KT = cfg.B, cfg.S, cfg.D, cfg.T, cfg.KT
    if cfg.mm_dt == "bf16":
        import ml_dtypes
        mmnp = ml_dtypes.bfloat16
    else:
        mmnp = np.float32
    x = np.asarray(in_features, dtype=np.float32).reshape(T, D)
    # xT[p, kt, t] = x[t, kt*128 + p]
    xT = np.ascontiguousarray(
        x.T.reshape(KT, 128, T).transpose(1, 0, 2))
    mask = np.triu(np.ones((128, 128), dtype=np.float32))
    wq = np.asarray(weight_q, dtype=np.float32) * (1.0 / np.sqrt(cfg.HD))
    wk = np.asarray(weight_k, dtype=np.float32)
    wv = np.asarray(weight_v, dtype=np.float32)
    wo = np.asarray(weight_o, dtype=np.float32)

    def wslice(w, c):
        # [128, KT, 128]: ws[p, kt, m] = w[c*128 + m, kt*128 + p]
        ws = w[c * 128:(c + 1) * 128, :]                  # [128, D]
        return np.ascontiguousarray(
            ws.T.reshape(KT, 128, 128).transpose(1, 0, 2))

    xT = xT.astype(mmnp)
    in_maps = []
    for c in range(n_cores):
        in_maps.append({
            "xT": xT,
            "wq": wslice(wq, c).astype(mmnp),
            "wk": wslice(wk, c).astype(mmnp),
            "wv": wslice(wv, c).astype(mmnp),
            "wo": np.ascontiguousarray(
                wo[:, c * 128:(c + 1) * 128].T).astype(mmnp),
            "mask": mask.astype(mmnp),
            "ident": np.eye(128, dtype=mmnp),
        })
    return in_maps


_CACHE = {}


def _get_program(cfg: Cfg):
    key = (cfg.B, cfg.S, cfg.D, cfg.TCH, cfg.QCH, cfg.mm_dt)
    if key not in _CACHE:
        _CACHE[key] = build_program(cfg)
    return _CACHE[key]


def run(inputs, cfg: Cfg, trace=False, trace_kwargs=None):
    import time
    from concourse.bass_utils import run_bass_kernel_spmd
    nc = _get_program(cfg)
    in_maps = prep_inputs(**inputs, cfg=cfg)
    last = None
    for attempt in range(3):
        try:
            res = run_bass_kernel_spmd(
                nc, in_maps, core_ids=list(range(8)), trace=trace,
                **(trace_kwargs or {}))
            break
        except Exception as e:  # transient NRT device wedges happen
            last = e
            time.sleep(10)
    else:
        raise last
    parts = [np.asarray(r["out_p"], dtype=np.float32) for r in res.results]
    out = np.sum(np.stack(parts, 0), axis=0)
    return out.astype(np.float32).reshape(cfg.B, cfg.S, cfg.D), res


def kernel(in_features, weight_q, weight_k, weight_v, weight_o):
    cfg = Cfg()
    out, _ = run(dict(in_features=in_features, weight_q=weight_q,
                      weight_k=weight_k, weight_v=weight_v,
                      weight_o=weight_o), cfg)
    return out
